# revision 66
# baseline (speedup 1.0000x reference)
"""Distributed Trainium2 Bass kernel for nn_Attention_66915590471696.

Sharding: 8 cores, each core owns 2 heads (core c -> heads 2c, 2c+1) and
processes BOTH batches.  The out-projection is computed per-core against the
owned head rows of Wout; the host sums the 8 partial outputs.

v3 (vs v2 baseline at ~492us):
  - Two heads' score matmuls are ROW-TILED CONCURRENT (h0 on PE rows 0-63,
    h1 on rows 64-127) into one 2-bank [128,1024] PSUM tile -> ONE exp and
    ONE bias-mult per (b,jt) pair.  Halves the ACT/DVE instruction count that
    was starving the PE and causing permanent HAM down-throttle (1.2GHz).
  - bias tiles hold h0|h1 side by side and are shared by both batches:
    bias DMA traffic halves to 8.4MB/core.
  - va is [j, v(64) | ones(64)] (M=128): the softmax denominator comes out
    of the oacc matmul replicated on PSUM partitions 64:128, so the
    normalizer is a straight DVE reciprocal_approx_fast + tensor_tensor.
    Kills the 32 `1*64*512` broadcast matmuls (28.5us of PE).
  - output written in bf16 (halves outbound DMA; host sums partials in f32).
"""
import sys, os, types, math
sys.path.insert(0, '/opt/trn_rl_repo')
import numpy as np
from contextlib import ExitStack
from collections import deque


def _install_axon_hooks_shim():
    try:
        import antenv.axon_hooks  # noqa
        return
    except ImportError:
        pass
    try:
        from trn_agent_boot.trn_boot import _ntff_profile_via_ctypes
        hook = _ntff_profile_via_ctypes('/opt/axon/libaxon_pjrt.so')
    except Exception:
        hook = None
    mod = types.ModuleType('antenv.axon_hooks')
    mod._hook = hook
    mod.get_axon_ntff_profile_hook = lambda: mod._hook
    def set_axon_ntff_profile_hook(h):
        mod._hook = h
    mod.set_axon_ntff_profile_hook = set_axon_ntff_profile_hook
    sys.modules['antenv.axon_hooks'] = mod


_install_axon_hooks_shim()

import concourse.bass as bass
import concourse.tile as tile
from concourse import mybir, bacc
from concourse.masks import make_identity

F32 = mybir.dt.float32
BF16 = mybir.dt.bfloat16

B, N, D, H, DH = 2, 2048, 1024, 16, 64
P = 128
NH = 2               # heads per core
NC = 8               # cores
SCALE = DH ** -0.5
NCH = N // P         # 16 n-chunks
JT = N // P          # 16 j tiles
IQ = 4               # i chunks
IQW = N // IQ        # 512


def build_nc():
    nc = bacc.Bacc("TRN2", target_bir_lowering=False, debug=False)

    xt = nc.declare_dram_parameter("xt", [B, P, D // P, N], BF16, isOutput=False)
    wq = nc.declare_dram_parameter("wq", [P, D // P, P], BF16, isOutput=False)
    wk = nc.declare_dram_parameter("wk", [P, D // P, P], BF16, isOutput=False)
    wv = nc.declare_dram_parameter("wv", [P, D // P, P], BF16, isOutput=False)
    wmix = nc.declare_dram_parameter("wmix", [P, D // P, NH], BF16, isOutput=False)
    wout = nc.declare_dram_parameter("wout", [P, D], BF16, isOutput=False)
    rott = nc.declare_dram_parameter("rott", [DH, N], F32, isOutput=False)
    # [jt, iq, 128(j), h0-block | h1-block] with exp() pre-applied on the host
    biasP = nc.declare_dram_parameter("biasP", [JT, IQ, P, NH * IQW], BF16, isOutput=False)
    vrp = nc.declare_dram_parameter("vrp", [B, NH, P, NCH, DH], BF16, isOutput=False)
    out = nc.declare_dram_parameter("out", [B, NCH, P, D], BF16, isOutput=True)

    with tile.TileContext(nc) as tc:
        with ExitStack() as ctx:
            consts = ctx.enter_context(tc.tile_pool(name="consts", bufs=1))
            wpool = ctx.enter_context(tc.tile_pool(name="wpool", bufs=1))
            proj = ctx.enter_context(tc.tile_pool(name="proj", bufs=1))
            # holds batch-1 tensors consumed AFTER the transient pools close:
            # b1's rope + v-lerp run interleaved with b0's first attention
            # pass, filling the PE/ACT idle window at the end of phase B.
            blate = ctx.enter_context(tc.tile_pool(name="blate", bufs=1))

            # ---- weights + x DMAs first (front of the DMA queues); only
            # wq + the first x chunk gate the first projection MM ----
            wq_t = wpool.tile([P, D // P, P], BF16)
            nc.sync.dma_start(wq_t[:], wq[:])

            # attention stream pools allocated BEFORE the transient x pools so
            # bias tiles never overlap the x region.
            biasb = ctx.enter_context(tc.tile_pool(name="biasb", bufs=8))
            esp = ctx.enter_context(tc.tile_pool(name="esp", bufs=5, side="right"))
            ptp = ctx.enter_context(tc.tile_pool(name="ptp", bufs=6))

            # bias tiles stream in consumption order on the sync HWDGE
            # queue, behind the x/w/vr loads.  iq0's tiles are loaded TWICE
            # (pass1 for b0, pass2 for b1) so the ring stays at 10 while the
            # two passes run ~20us apart; each tile has exactly one reader.
            PF = 8
            # iq0 and the final iq are split into per-batch passes; their
            # bias tiles appear twice in the stream (re-DMA'd, single-reader)
            bias_seq = ([(jt, 0) for jt in range(JT)] * 2 +
                        [(jt, iq) for iq in range(1, IQ) for jt in range(JT)] +
                        [(jt, IQ - 1) for jt in range(JT)])
            bias_tiles = []

            def issue_bias(n):
                if n >= len(bias_seq):
                    return
                jt_, iq_ = bias_seq[n]
                t = biasb.tile([P, NH * IQW], BF16, tag="bias", name=f"bias{n}")
                nc.sync.dma_start(t[:], biasP[jt_, iq_])
                bias_tiles.append(t)

            pctx = ExitStack()
            xpool = pctx.enter_context(tc.tile_pool(name="xpool", bufs=1))
            ptmp = pctx.enter_context(tc.tile_pool(name="ptmp", bufs=1))
            psB = pctx.enter_context(tc.tile_pool(name="psB", bufs=1, space="PSUM"))

            # x first, split in quarter-D chunks so the projections can start
            # as soon as the first chunk lands (PSUM accumulation spans all)
            x_t = [None, None]
            x_t[0] = xpool.tile([P, D // P, N], BF16, tag="xt0", name="x_t0")
            nc.sync.dma_start(x_t[0][:, 0:1], xt[0][:, 0:1])
            wk_t = wpool.tile([P, D // P, P], BF16)
            nc.sync.dma_start(wk_t[:], wk[:])
            wv_t = wpool.tile([P, D // P, P], BF16)
            nc.sync.dma_start(wv_t[:], wv[:])
            wmix_t = wpool.tile([P, D // P, NH], BF16)
            nc.sync.dma_start(wmix_t[:], wmix[:])
            for lo, hi in ((1, 2), (2, 4), (4, 6), (6, 8)):
                nc.sync.dma_start(x_t[0][:, lo:hi], xt[0][:, lo:hi])
            rt = [None, None]
            for rih in range(2):
                rsl = slice(rih * (N // 2), (rih + 1) * (N // 2))
                rt[rih] = ptmp.tile([DH, N // 2], F32, tag="rt", name=f"rt{rih}")
                nc.sync.dma_start(rt[rih][:], rott[:, rsl])
            x_t[1] = xpool.tile([P, D // P, N], BF16, tag="xt1", name="x_t1")
            for ch in range(4):
                nc.sync.dma_start(x_t[1][:, 2 * ch:2 * ch + 2], xt[1][:, 2 * ch:2 * ch + 2])
            # wout is only needed by the out-projection (~150us in)
            wout_t = wpool.tile([P, D], BF16)
            nc.sync.dma_start(wout_t[:], wout[:])

            # value_residual early (unblocks v_aug right after projections);
            # b1's tiles live in blate (consumed during the late lerp)
            vr_t = {}
            for b in range(B):
                for hh in range(NH):
                    vr_t[(b, hh)] = (blate if b == 1 else ptmp).tile(
                        [P, NCH, DH], BF16, tag=f"vr{b}{hh}", name=f"vr{b}{hh}")
                    nc.sync.dma_start(vr_t[(b, hh)][:], vrp[b, hh])

            # prefetch the bias stream behind x/vr
            for n in range(PF):
                issue_bias(n)

            # ---- constants ----
            ident_b = consts.tile([P, P], BF16)
            make_identity(nc, ident_b[:])

            # rotary -> cosT / sinT_rot [128, N] bf16 (head-duplicated on
            # partitions; sin built in place, low half of each 64-block
            # negated)
            sinT_rot = consts.tile([P, N], BF16)
            cosT = consts.tile([P, N], BF16)
            for rih in range(2):
                rsl = slice(rih * (N // 2), (rih + 1) * (N // 2))
                wrap = ptmp.tile([DH, N // 2], F32, tag="wrap")
                nc.vector.add_range_wrap(wrap[:], rt[rih][:], 0.0, math.pi, 2 * math.pi)
                nc.scalar.activation(sinT_rot[0:DH, rsl], wrap[:], mybir.ActivationFunctionType.Sin)
                wrap2 = ptmp.tile([DH, N // 2], F32, tag="wrap")
                nc.vector.add_range_wrap(wrap2[:], rt[rih][:], math.pi / 2, math.pi, 2 * math.pi)
                nc.scalar.activation(cosT[0:DH, rsl], wrap2[:], mybir.ActivationFunctionType.Sin)
            nc.vector.tensor_copy(sinT_rot[DH:P, :], sinT_rot[0:DH, :])
            nc.vector.tensor_copy(cosT[DH:P, :], cosT[0:DH, :])
            for lo in (0, DH):
                nc.vector.tensor_scalar(sinT_rot[lo:lo + 32, :], sinT_rot[lo:lo + 32, :],
                                        -1.0, None, mybir.AluOpType.mult)

            # ---- projections (both batches) ----
            # b0 gets its full chain (proj + rope + lerp) inside the
            # transient scope; b1 gets only the PE work (proj MMs +
            # transposes, with vps copied to SBUF) -- its rope and lerp are
            # deferred into the b0-only first attention pass.
            qt = [None, None]; kt = [None, None]
            mixn = [None, None]; mixc_l = [None, None]
            vt = [None, None]
            vaug = {}
            qt_raw1 = kt_raw1 = None
            vpsS = [blate.tile([P, NCH, DH], BF16, tag=f"vpsS{hh}", name=f"vpsS{hh}")
                    for hh in range(NH)]
            for b in range(B):
                late = (b == 1)
                qt_raw = (blate if late else ptmp).tile([P, N], BF16, tag=f"qt_raw{b}",
                                                        name=f"qt_raw{b}")
                kt_raw = (blate if late else ptmp).tile([P, N], BF16, tag=f"kt_raw{b}",
                                                        name=f"kt_raw{b}")
                vt[b] = ptmp.tile([P, N], BF16, tag=f"vt{b}", name=f"vt{b}")
                mixT = ptmp.tile([NH, N], BF16, tag="mixT", name=f"mixT{b}")
                specs = [("q", wq_t, P, qt_raw), ("k", wk_t, P, kt_raw),
                         ("v", wv_t, P, vt[b]), ("m", wmix_t, NH, mixT)]
                for name, w_t, M, dst in specs:
                    pps4 = [psB.tile([P, IQW], F32, tag="S", name=f"pp{b}{name}{c}",
                                     bufs=6)
                            for c in range(IQ)]
                    for kk in range(D // P):
                        for c in range(IQ):
                            nc.tensor.matmul(
                                pps4[c][:M, :], w_t[:, kk, :M],
                                x_t[b][:, kk, c * IQW:(c + 1) * IQW],
                                start=(kk == 0), stop=(kk == D // P - 1))
                    for c in range(IQ):
                        sl = slice(c * IQW, (c + 1) * IQW)
                        if name == "q":
                            nc.scalar.mul(dst[:, sl], pps4[c][:, :], SCALE)
                        elif name == "m":
                            nc.scalar.activation(dst[:NH, sl], pps4[c][:NH, :],
                                                 mybir.ActivationFunctionType.Sigmoid)
                        else:
                            nc.scalar.copy(dst[:, sl], pps4[c][:, :])

                # RoPE on qT and kT (b0 now; b1 deferred)
                qt[b] = proj.tile([P, N], BF16, tag=f"qt{b}", name=f"qt{b}")
                kt[b] = proj.tile([P, N], BF16, tag=f"kt{b}", name=f"kt{b}")
                if late:
                    qt_raw1, kt_raw1 = qt_raw, kt_raw
                else:
                    for src, dst in ((qt_raw, qt[b]), (kt_raw, kt[b])):
                        rot_t = ptmp.tile([P, N], BF16, tag="rot")
                        for hh in range(NH):
                            lo = hh * DH
                            nc.vector.tensor_copy(rot_t[lo:lo + 32, :], src[lo + 32:lo + 64, :])
                            nc.vector.tensor_copy(rot_t[lo + 32:lo + 64, :], src[lo:lo + 32, :])
                        nc.vector.tensor_tensor(dst[:], src[:], cosT[:], mybir.AluOpType.mult)
                        nc.vector.tensor_tensor(rot_t[:], rot_t[:], sinT_rot[:], mybir.AluOpType.mult)
                        nc.vector.tensor_tensor(dst[:], dst[:], rot_t[:], mybir.AluOpType.add)

                # mix natural [128, NCH, NH] f32 via PE transposes; mixc = 1-mix
                mixn[b] = proj.tile([P, NCH, NH], F32, tag=f"mixn{b}", name=f"mixn{b}")
                mixc = proj.tile([P, NCH, NH], F32, tag=f"mixc{b}", name=f"mixc{b}")
                mixc_l[b] = mixc
                for t in range(NCH):
                    mps = psB.tile([P, DH], BF16, tag="T", bufs=2)
                    nc.tensor.matmul(mps[:, :NH], mixT[:NH, t * P:(t + 1) * P], ident_b[:NH, :NH],
                                     is_transpose=True, start=True, stop=True)
                    nc.scalar.copy(mixn[b][:, t, :], mps[:, :NH])
                nc.vector.tensor_scalar(mixc[:], mixn[b][:], -1.0, 1.0,
                                        mybir.AluOpType.mult, mybir.AluOpType.add)

                # v_aug: [j, ones(64) | v(64)]; the ones block replicates the
                # softmax denominator onto PSUM partitions 0:64 of oacc (the
                # custom reciprocal DVE op requires a base-0 input on HW).
                for hh in range(NH):
                    va = proj.tile([P, NCH, P], BF16, tag=f"va{b}{hh}", name=f"va{b}{hh}")
                    nc.gpsimd.memset(va[:, :, 0:DH], 1.0)
                    vr = vr_t[(b, hh)]
                    for t in range(NCH):
                        vps = psB.tile([P, DH], BF16, tag="T", bufs=2)
                        lo = hh * DH
                        nc.tensor.matmul(vps[:, :DH], vt[b][lo:lo + DH, t * P:(t + 1) * P],
                                         ident_b[lo:lo + DH, lo:lo + DH], is_transpose=True,
                                         start=True, stop=True)
                        if late:
                            # park the transposed v in SBUF; lerp runs later.
                            # On DVE: these copies are the last readers of the
                            # psB banks, and the ACT queue (q/k/v copies)
                            # drains ~10us later than the DVE here -- putting
                            # them on ACT delayed the attention-pool barrier
                            # and the first exp to ~104us (measured).
                            nc.vector.tensor_copy(vpsS[hh][:, t, :], vps[:, :DH])
                        else:
                            # va = v*(1-mix) + vr*mix
                            vrm = ptmp.tile([P, DH], BF16, tag="df")
                            nc.vector.tensor_scalar(vrm[:], vr[:, t, :],
                                                    mixn[b][:, t, hh:hh + 1],
                                                    None, mybir.AluOpType.mult)
                            nc.vector.scalar_tensor_tensor(va[:, t, DH:P], vps[:, :DH],
                                                           mixc[:, t, hh:hh + 1], vrm[:],
                                                           mybir.AluOpType.mult, mybir.AluOpType.add)
                    vaug[(b, hh)] = va
            pctx.close()

            # ---- b1's deferred rope + lerp, emitted as small thunks woven
            # into the b0-only first attention pass (fills the DVE while the
            # PE/ACT stream b0's scores+exp) ----
            def late_ops_gen():
                for src, dst in ((qt_raw1, qt[1]), (kt_raw1, kt[1])):
                    rot_t = blate.tile([P, N], BF16, tag="rot1", bufs=1,
                                       name=f"rot1_{src.name}")
                    for hh in range(NH):
                        lo = hh * DH
                        yield lambda r=rot_t, s=src, lo=lo: nc.vector.tensor_copy(
                            r[lo:lo + 32, :], s[lo + 32:lo + 64, :])
                        yield lambda r=rot_t, s=src, lo=lo: nc.vector.tensor_copy(
                            r[lo + 32:lo + 64, :], s[lo:lo + 32, :])
                    yield lambda d=dst, s=src: nc.vector.tensor_tensor(
                        d[:], s[:], cosT[:], mybir.AluOpType.mult)
                    yield lambda r=rot_t: nc.vector.tensor_tensor(
                        r[:], r[:], sinT_rot[:], mybir.AluOpType.mult)
                    yield lambda d=dst, r=rot_t: nc.vector.tensor_tensor(
                        d[:], d[:], r[:], mybir.AluOpType.add)
                for hh in range(NH):
                    for t in range(NCH):
                        def lerp(hh=hh, t=t):
                            vrm = blate.tile([P, DH], BF16, tag="vrm", bufs=2,
                                             name=f"vrm1_{hh}_{t}")
                            nc.vector.tensor_scalar(vrm[:], vr_t[(1, hh)][:, t, :],
                                                    mixn[1][:, t, hh:hh + 1],
                                                    None, mybir.AluOpType.mult)
                            nc.vector.scalar_tensor_tensor(
                                vaug[(1, hh)][:, t, DH:P], vpsS[hh][:, t, :],
                                mixc_l[1][:, t, hh:hh + 1], vrm[:],
                                mybir.AluOpType.mult, mybir.AluOpType.add)
                        yield lerp

            late_ops = deque(late_ops_gen())

            # ---- tail pools (allocated after x/transients are freed) ----
            # all 8 outTq tiles (2 per iq) stay live until the outproj tail
            otqp = ctx.enter_context(tc.tile_pool(name="otqp", bufs=8))
            finp = ctx.enter_context(tc.tile_pool(name="finp", bufs=3))
            zpool = ctx.enter_context(tc.tile_pool(name="zpool", bufs=4, side="right"))
            # attention PSUM pools live in their own stack so the outproj
            # tail can reclaim all 8 banks for a deeper ring
            actx = ExitStack()
            ps = actx.enter_context(tc.tile_pool(name="ps", bufs=2, space="PSUM"))
            oaccp = actx.enter_context(tc.tile_pool(name="oaccp", bufs=4, space="PSUM"))

            # ---- attention, streaming per i-chunk of 512 ----
            # Per (b, jt): two row-tiled CONCURRENT score MMs (h0 rows 0-63,
            # h1 rows 64-127) -> one [128,1024] PSUM pair -> one exp -> one
            # bias-mult -> two oacc MMs.  oacc rows 64:128 accumulate the
            # denominator via va's ones block.
            # oacc MMs trail the scores stream by DEPTH/2 (b,jt) groups so
            # their pT operand is long-ready when the PE reaches them; a
            # shallow trail head-of-line-blocks the next S-pair behind an
            # oacc waiting on exp+mult.  pT ring (6) bounds the trail.
            DEPTH = 8
            pending = deque()   # (oacc_tile, va, jt, pT, half)

            def flush_one():
                oa, va, jt_, pT_, half = pending.popleft()
                nc.tensor.matmul(oa[:], va[:, jt_, :], pT_[:, half * IQW:(half + 1) * IQW],
                                 start=(jt_ == 0), stop=(jt_ == JT - 1))

            def emit_norm_one(oa, otq_b, hh):
                # rz = 1/z from the replicated denominator rows (0:64); then
                # scale the v rows (64:128).
                rz = zpool.tile([DH, IQW], F32, tag="rz")
                nc.vector.reciprocal_approx_fast(rz[:], oa[0:DH, :])
                nc.vector.tensor_tensor(otq_b[hh * DH:(hh + 1) * DH, :], rz[:],
                                        oa[DH:P, :], mybir.AluOpType.mult)

            def emit_proj_unit(iq, b, it, outTq, pool, split):
                tg = iq * (IQW // P) + it
                pp = pool.tile([P, 2 * IQW], F32, tag="S", name=f"pp{iq}{b}{it}")
                for dfi in range(2):
                    nc.tensor.matmul(pp[:, dfi * IQW:(dfi + 1) * IQW],
                                     outTq[b][:, it * P:(it + 1) * P],
                                     wout_t[:, dfi * IQW:(dfi + 1) * IQW],
                                     start=True, stop=True)
                # fin copy (GPSIMD cannot read PSUM): in-stream units keep
                # the ACT queue clean (exp cadence) -> single DVE cast; tail
                # units split DVE/ACT halves to halve the tail latency.
                fin = finp.tile([P, D], BF16, tag="fin")
                if split:
                    nc.vector.tensor_copy(fin[:, 0:IQW], pp[:, 0:IQW])
                    nc.scalar.copy(fin[:, IQW:2 * IQW], pp[:, IQW:2 * IQW])
                else:
                    nc.vector.tensor_copy(fin[:], pp[:])
                nc.gpsimd.dma_start(out[b, tg], fin[:])

            norms_pending = deque()   # thunks; popped at jt 0/1 half-steps
            proj_units = deque()      # (iq, b, it, outTq)

            def emit_attn(iq, jt, b, seqbase, oacc_b, outTq_b):
                isl = slice(iq * IQW, (iq + 1) * IQW)
                bias_sb = bias_tiles[seqbase + jt]
                S = ps.tile([P, 2 * IQW], F32, tag="S", name=f"S{iq}_{jt}{b}")
                for hh in range(NH):
                    lo = hh * DH
                    nc.tensor.matmul(S[:, hh * IQW:(hh + 1) * IQW],
                                     kt[b][lo:lo + DH, jt * P:(jt + 1) * P],
                                     qt[b][lo:lo + DH, isl],
                                     start=True, stop=True)
                eS = esp.tile([P, 2 * IQW], BF16, tag="eS")
                nc.scalar.activation(eS[:], S[:], mybir.ActivationFunctionType.Exp)
                pT = ptp.tile([P, 2 * IQW], BF16, tag="pT")
                nc.vector.tensor_tensor(pT[:], bias_sb[:], eS[:], mybir.AluOpType.mult)
                for hh in range(NH):
                    pending.append((oacc_b[hh], vaug[(b, hh)], jt, pT, hh))
                while len(pending) > DEPTH:
                    flush_one()

            def push_norms(oacc_b, outTq_b):
                for hh in range(NH):
                    norms_pending.append(
                        lambda oa=oacc_b[hh], oq=outTq_b, hh=hh: emit_norm_one(oa, oq, hh))

            for iq in range(IQ):
                outTq = [otqp.tile([P, IQW], BF16, tag="otq", name=f"otq{iq}_{b}")
                         for b in range(B)]
                if iq == 0:
                    # pass1: b0-only attention over iq0, woven with b1's
                    # deferred rope/lerp DVE thunks
                    oacc_b0 = [oaccp.tile([P, IQW], F32, tag="oacc",
                                          name=f"oacc0_0{hh}") for hh in range(NH)]
                    for jt in range(JT):
                        issue_bias(jt + PF)
                        emit_attn(0, jt, 0, 0, oacc_b0, outTq[0])
                        for _ in range(3):
                            if late_ops:
                                late_ops.popleft()()
                    while pending:
                        flush_one()
                    while late_ops:
                        late_ops.popleft()()
                    push_norms(oacc_b0, outTq[0])
                    # pass2: b1 over iq0 (bias tiles re-streamed, seq 16..31)
                    oacc_b1 = [oaccp.tile([P, IQW], F32, tag="oacc",
                                          name=f"oacc0_1{hh}") for hh in range(NH)]
                    for jt in range(JT):
                        issue_bias(JT + jt + PF)
                        emit_attn(0, jt, 1, JT, oacc_b1, outTq[1])
                        if jt in (0, 1) and norms_pending:
                            norms_pending.popleft()()
                    while pending:
                        flush_one()
                    push_norms(oacc_b1, outTq[1])
                    oacc_pair = [oacc_b0, oacc_b1]
                elif iq < IQ - 1:
                    oacc_pair = [[oaccp.tile([P, IQW], F32, tag="oacc",
                                             name=f"oacc{iq}_{b}{hh}") for hh in range(NH)]
                                 for b in range(B)]
                    seqbase = JT + iq * JT
                    for jt in range(JT):
                        issue_bias(seqbase + jt + PF)
                        for b in range(B):
                            emit_attn(iq, jt, b, seqbase, oacc_pair[b], outTq[b])
                            if jt in (0, 1) and norms_pending:
                                norms_pending.popleft()()
                        # previous iq's out-projection rides the attention
                        # stream (one unit per jt) so its out-DMA overlaps
                        # compute instead of piling up in the tail
                        if jt >= 2 and proj_units and proj_units[0][0] < iq:
                            emit_proj_unit(*proj_units.popleft(), pool=ps, split=False)
                    while pending:
                        flush_one()
                    for b in range(B):
                        push_norms(oacc_pair[b], outTq[b])
                else:
                    # final iq split by batch: b0's norms + out-projection
                    # drain during b1's pass, shrinking the serial tail
                    oacc_b0 = [oaccp.tile([P, IQW], F32, tag="oacc",
                                          name=f"oacc{iq}_0{hh}") for hh in range(NH)]
                    seqbase = JT + iq * JT
                    for jt in range(JT):
                        issue_bias(seqbase + jt + PF)
                        emit_attn(iq, jt, 0, seqbase, oacc_b0, outTq[0])
                        if jt in (0, 1, 2, 3) and norms_pending:
                            norms_pending.popleft()()
                        if jt >= 4 and proj_units and proj_units[0][0] < iq:
                            emit_proj_unit(*proj_units.popleft(), pool=ps, split=False)
                    while pending:
                        flush_one()
                    push_norms(oacc_b0, outTq[0])
                    oacc_b1 = [oaccp.tile([P, IQW], F32, tag="oacc",
                                          name=f"oacc{iq}_1{hh}") for hh in range(NH)]
                    seqbase2 = JT + IQ * JT
                    for jt in range(JT):
                        issue_bias(seqbase2 + jt + PF)
                        emit_attn(iq, jt, 1, seqbase2, oacc_b1, outTq[1])
                        if jt in (0, 1) and norms_pending:
                            norms_pending.popleft()()
                        if jt == 2:
                            for it in range(IQW // P):
                                proj_units.append((iq, 0, it, outTq))
                        if jt >= 3 and proj_units:
                            emit_proj_unit(*proj_units.popleft(), pool=ps, split=False)
                    while pending:
                        flush_one()
                    push_norms(oacc_b1, outTq[1])
                    oacc_pair = [oacc_b0, oacc_b1]
                if iq < IQ - 1:
                    for b in range(B):
                        for it in range(IQW // P):
                            proj_units.append((iq, b, it, outTq))
                else:
                    for it in range(IQW // P):
                        proj_units.append((iq, 1, it, outTq))
            # all out-projections run as one pipelined tail: any unit placed
            # inside the attention stream steals an S-ring slot and stalls
            # the exp cadence ~1.2us (measured), 24x per run.  The attention
            # PSUM pools close first so the tail gets a 4-deep [128,1024]
            # ring (all 8 banks) -- with the 2-slot S-ring the tail ran
            # copy-serialized at ~2us/unit (65us, measured).
            while norms_pending:
                norms_pending.popleft()()
            actx.close()
            psT = ctx.enter_context(tc.tile_pool(name="psT", bufs=4, space="PSUM"))
            while proj_units:
                emit_proj_unit(*proj_units.popleft(), pool=psT, split=True)

    nc.compile()
    return nc


def make_in_maps(x, mask, rotary_emb, attn_bias, value_residual, Wq, Wkv, Wmix, Wout, bout):
    """Shard + lay out the full inputs for the 8 cores (bf16 staging)."""
    import ml_dtypes
    bf16 = ml_dtypes.bfloat16
    x = np.asarray(x); rotary_emb = np.asarray(rotary_emb)
    attn_bias = np.asarray(attn_bias); value_residual = np.asarray(value_residual)
    Wq = np.asarray(Wq); Wkv = np.asarray(Wkv); Wmix = np.asarray(Wmix)
    Wout = np.asarray(Wout); bout = np.asarray(bout)

    xt_pre = np.ascontiguousarray(
        x.transpose(0, 2, 1).reshape(B, D // P, P, N).transpose(0, 2, 1, 3)).astype(bf16)
    rott = np.ascontiguousarray(rotary_emb.T)

    def wslice(Wcols):  # [1024, 128 or NH] -> [128, 8, M]
        M = Wcols.shape[1]
        return np.ascontiguousarray(
            Wcols.reshape(D // P, P, M).transpose(1, 0, 2)).astype(bf16)

    in_maps = []
    for c in range(NC):
        h0 = NH * c
        hs = slice(h0, h0 + NH)
        # exp(bias) transposed to [h, j, i], then arranged so each (jt, iq)
        # tile is [128(j), h0-block(512) | h1-block(512)]
        biasT = np.exp(attn_bias[hs].transpose(0, 2, 1))  # [NH, j, i]
        biasPa = np.ascontiguousarray(
            biasT.reshape(NH, JT, P, IQ, IQW).transpose(1, 3, 2, 0, 4)
            .reshape(JT, IQ, P, NH * IQW)).astype(bf16)
        vrp = np.ascontiguousarray(
            value_residual[:, hs].reshape(B, NH, NCH, P, DH).transpose(0, 1, 3, 2, 4)).astype(bf16)
        in_maps.append({
            "xt": xt_pre,
            "wq": wslice(Wq[:, h0 * DH:(h0 + NH) * DH]),
            "wk": wslice(Wkv[:, h0 * DH:(h0 + NH) * DH]),
            "wv": wslice(Wkv[:, H * DH + h0 * DH: H * DH + (h0 + NH) * DH]),
            "wmix": wslice(Wmix[:, hs]),
            "wout": np.ascontiguousarray(Wout[h0 * DH:(h0 + NH) * DH, :]).astype(bf16),
            "rott": rott,
            "biasP": biasPa,
            "vrp": vrp,
        })
    return in_maps


def unshard(results, bout):
    full = np.zeros((B, NCH, P, D), np.float32)
    for r in results:
        full += r["out"].astype(np.float32)
    return full.reshape(B, N, D) + np.asarray(bout, np.float32)


_NC_CACHE = None


def kernel(**inputs):
    global _NC_CACHE
    from concourse.bass_utils import run_bass_kernel_spmd
    if _NC_CACHE is None:
        _NC_CACHE = build_nc()
    in_maps = make_in_maps(**inputs)
    res = run_bass_kernel_spmd(_NC_CACHE, in_maps, core_ids=list(range(NC)))
    return unshard(res.results, inputs["bout"])


# revision 67
# speedup vs baseline: 1.1861x; 1.1861x over previous
"""Distributed Trainium2 Bass kernel for nn_Attention_66915590471696.

Sharding: 8 cores, each core owns 2 heads (core c -> heads 2c, 2c+1) and
processes BOTH batches.  The out-projection is computed per-core against the
owned head rows of Wout; the host sums the 8 partial outputs.

v3 (vs v2 baseline at ~492us):
  - Two heads' score matmuls are ROW-TILED CONCURRENT (h0 on PE rows 0-63,
    h1 on rows 64-127) into one 2-bank [128,1024] PSUM tile -> ONE exp and
    ONE bias-mult per (b,jt) pair.  Halves the ACT/DVE instruction count that
    was starving the PE and causing permanent HAM down-throttle (1.2GHz).
  - bias tiles hold h0|h1 side by side and are shared by both batches:
    bias DMA traffic halves to 8.4MB/core.
  - va is [j, v(64) | ones(64)] (M=128): the softmax denominator comes out
    of the oacc matmul replicated on PSUM partitions 64:128, so the
    normalizer is a straight DVE reciprocal_approx_fast + tensor_tensor.
    Kills the 32 `1*64*512` broadcast matmuls (28.5us of PE).
  - output written in bf16 (halves outbound DMA; host sums partials in f32).
"""
import sys, os, types, math
sys.path.insert(0, '/opt/trn_rl_repo')
import numpy as np
from contextlib import ExitStack
from collections import deque


def _install_axon_hooks_shim():
    try:
        import antenv.axon_hooks  # noqa
        return
    except ImportError:
        pass
    try:
        from trn_agent_boot.trn_boot import _ntff_profile_via_ctypes
        hook = _ntff_profile_via_ctypes('/opt/axon/libaxon_pjrt.so')
    except Exception:
        hook = None
    mod = types.ModuleType('antenv.axon_hooks')
    mod._hook = hook
    mod.get_axon_ntff_profile_hook = lambda: mod._hook
    def set_axon_ntff_profile_hook(h):
        mod._hook = h
    mod.set_axon_ntff_profile_hook = set_axon_ntff_profile_hook
    sys.modules['antenv.axon_hooks'] = mod


_install_axon_hooks_shim()

import concourse.bass as bass
import concourse.tile as tile
from concourse import mybir, bacc
from concourse.masks import make_identity

F32 = mybir.dt.float32
BF16 = mybir.dt.bfloat16

B, N, D, H, DH = 2, 2048, 1024, 16, 64
P = 128
NH = 2               # heads per core
NC = 8               # cores
SCALE = DH ** -0.5
NCH = N // P         # 16 n-chunks
JT = N // P          # 16 j tiles
IQ = 4               # i chunks
IQW = N // IQ        # 512


def build_nc():
    nc = bacc.Bacc("TRN2", target_bir_lowering=False, debug=False)

    xt = nc.declare_dram_parameter("xt", [B, P, D // P, N], BF16, isOutput=False)
    wq = nc.declare_dram_parameter("wq", [P, D // P, P], BF16, isOutput=False)
    wk = nc.declare_dram_parameter("wk", [P, D // P, P], BF16, isOutput=False)
    wv = nc.declare_dram_parameter("wv", [P, D // P, P], BF16, isOutput=False)
    wmix = nc.declare_dram_parameter("wmix", [P, D // P, NH], BF16, isOutput=False)
    wout = nc.declare_dram_parameter("wout", [P, D], BF16, isOutput=False)
    rott = nc.declare_dram_parameter("rott", [DH, N], F32, isOutput=False)
    # [jt, iq, 128(j), h0-block | h1-block] with exp() pre-applied on the host
    biasP = nc.declare_dram_parameter("biasP", [JT, IQ, P, NH * IQW], BF16, isOutput=False)
    vrp = nc.declare_dram_parameter("vrp", [B, NH, P, NCH, DH], BF16, isOutput=False)
    out = nc.declare_dram_parameter("out", [B, NCH, P, D], BF16, isOutput=True)

    with tile.TileContext(nc) as tc:
        with ExitStack() as ctx:
            consts = ctx.enter_context(tc.tile_pool(name="consts", bufs=1))
            wpool = ctx.enter_context(tc.tile_pool(name="wpool", bufs=1))
            proj = ctx.enter_context(tc.tile_pool(name="proj", bufs=1))
            # holds batch-1 tensors consumed AFTER the transient pools close:
            # b1's rope + v-lerp run interleaved with b0's first attention
            # pass, filling the PE/ACT idle window at the end of phase B.
            blate = ctx.enter_context(tc.tile_pool(name="blate", bufs=1))

            # ---- weights + x DMAs first (front of the DMA queues); only
            # wq + the first x chunk gate the first projection MM ----
            wq_t = wpool.tile([P, D // P, P], BF16)
            nc.sync.dma_start(wq_t[:], wq[:])

            # attention stream pools allocated BEFORE the transient x pools so
            # bias tiles never overlap the x region.
            biasb = ctx.enter_context(tc.tile_pool(name="biasb", bufs=8))
            esp = ctx.enter_context(tc.tile_pool(name="esp", bufs=5, side="right"))
            ptp = ctx.enter_context(tc.tile_pool(name="ptp", bufs=6))

            # bias tiles stream in consumption order on the sync HWDGE
            # queue, behind the x/w/vr loads.  iq0's tiles are loaded TWICE
            # (pass1 for b0, pass2 for b1) so the ring stays at 10 while the
            # two passes run ~20us apart; each tile has exactly one reader.
            PF = 8
            # iq0 and the final iq are split into per-batch passes; their
            # bias tiles appear twice in the stream (re-DMA'd, single-reader)
            bias_seq = ([(jt, 0) for jt in range(JT)] * 2 +
                        [(jt, iq) for iq in range(1, IQ) for jt in range(JT)] +
                        [(jt, IQ - 1) for jt in range(JT)])
            bias_tiles = []

            def issue_bias(n):
                if n >= len(bias_seq):
                    return
                jt_, iq_ = bias_seq[n]
                t = biasb.tile([P, NH * IQW], BF16, tag="bias", name=f"bias{n}")
                nc.sync.dma_start(t[:], biasP[jt_, iq_])
                bias_tiles.append(t)

            pctx = ExitStack()
            xpool = pctx.enter_context(tc.tile_pool(name="xpool", bufs=1))
            ptmp = pctx.enter_context(tc.tile_pool(name="ptmp", bufs=1))
            psB = pctx.enter_context(tc.tile_pool(name="psB", bufs=1, space="PSUM"))

            # x first, split in quarter-D chunks so the projections can start
            # as soon as the first chunk lands (PSUM accumulation spans all)
            x_t = [None, None]
            x_t[0] = xpool.tile([P, D // P, N], BF16, tag="xt0", name="x_t0")
            nc.sync.dma_start(x_t[0][:, 0:1], xt[0][:, 0:1])
            wk_t = wpool.tile([P, D // P, P], BF16)
            nc.sync.dma_start(wk_t[:], wk[:])
            wv_t = wpool.tile([P, D // P, P], BF16)
            nc.sync.dma_start(wv_t[:], wv[:])
            wmix_t = wpool.tile([P, D // P, NH], BF16)
            nc.sync.dma_start(wmix_t[:], wmix[:])
            for lo, hi in ((1, 2), (2, 4), (4, 6), (6, 8)):
                nc.sync.dma_start(x_t[0][:, lo:hi], xt[0][:, lo:hi])
            rt = [None, None]
            for rih in range(2):
                rsl = slice(rih * (N // 2), (rih + 1) * (N // 2))
                rt[rih] = ptmp.tile([DH, N // 2], F32, tag="rt", name=f"rt{rih}")
                nc.sync.dma_start(rt[rih][:], rott[:, rsl])
            x_t[1] = xpool.tile([P, D // P, N], BF16, tag="xt1", name="x_t1")
            for ch in range(4):
                nc.sync.dma_start(x_t[1][:, 2 * ch:2 * ch + 2], xt[1][:, 2 * ch:2 * ch + 2])
            # wout is only needed by the out-projection (~150us in)
            wout_t = wpool.tile([P, D], BF16)
            nc.sync.dma_start(wout_t[:], wout[:])

            # value_residual early (unblocks v_aug right after projections);
            # b1's tiles live in blate (consumed during the late lerp)
            vr_t = {}
            for b in range(B):
                for hh in range(NH):
                    vr_t[(b, hh)] = (blate if b == 1 else ptmp).tile(
                        [P, NCH, DH], BF16, tag=f"vr{b}{hh}", name=f"vr{b}{hh}")
                    nc.sync.dma_start(vr_t[(b, hh)][:], vrp[b, hh])

            # prefetch the bias stream behind x/vr
            for n in range(PF):
                issue_bias(n)

            # ---- constants ----
            ident_b = consts.tile([P, P], BF16)
            make_identity(nc, ident_b[:])

            # rotary -> cosT / sinT_rot [128, N] bf16 (head-duplicated on
            # partitions; sin built in place, low half of each 64-block
            # negated)
            sinT_rot = consts.tile([P, N], BF16)
            cosT = consts.tile([P, N], BF16)
            for rih in range(2):
                rsl = slice(rih * (N // 2), (rih + 1) * (N // 2))
                wrap = ptmp.tile([DH, N // 2], F32, tag="wrap")
                nc.vector.add_range_wrap(wrap[:], rt[rih][:], 0.0, math.pi, 2 * math.pi)
                nc.scalar.activation(sinT_rot[0:DH, rsl], wrap[:], mybir.ActivationFunctionType.Sin)
                wrap2 = ptmp.tile([DH, N // 2], F32, tag="wrap")
                nc.vector.add_range_wrap(wrap2[:], rt[rih][:], math.pi / 2, math.pi, 2 * math.pi)
                nc.scalar.activation(cosT[0:DH, rsl], wrap2[:], mybir.ActivationFunctionType.Sin)
            nc.vector.tensor_copy(sinT_rot[DH:P, :], sinT_rot[0:DH, :])
            nc.vector.tensor_copy(cosT[DH:P, :], cosT[0:DH, :])
            for lo in (0, DH):
                nc.vector.tensor_scalar(sinT_rot[lo:lo + 32, :], sinT_rot[lo:lo + 32, :],
                                        -1.0, None, mybir.AluOpType.mult)

            # ---- projections (both batches) ----
            # b0 gets its full chain (proj + rope + lerp) inside the
            # transient scope; b1 gets only the PE work (proj MMs +
            # transposes, with vps copied to SBUF) -- its rope and lerp are
            # deferred into the b0-only first attention pass.
            qt = [None, None]; kt = [None, None]
            mixn = [None, None]; mixc_l = [None, None]
            vt = [None, None]
            vaug = {}
            qt_raw1 = kt_raw1 = None
            vpsS = [blate.tile([P, NCH, DH], BF16, tag=f"vpsS{hh}", name=f"vpsS{hh}")
                    for hh in range(NH)]
            for b in range(B):
                late = (b == 1)
                qt_raw = (blate if late else ptmp).tile([P, N], BF16, tag=f"qt_raw{b}",
                                                        name=f"qt_raw{b}")
                kt_raw = (blate if late else ptmp).tile([P, N], BF16, tag=f"kt_raw{b}",
                                                        name=f"kt_raw{b}")
                vt[b] = ptmp.tile([P, N], BF16, tag=f"vt{b}", name=f"vt{b}")
                mixT = ptmp.tile([NH, N], BF16, tag="mixT", name=f"mixT{b}")
                specs = [("q", wq_t, P, qt_raw), ("k", wk_t, P, kt_raw),
                         ("v", wv_t, P, vt[b]), ("m", wmix_t, NH, mixT)]
                for name, w_t, M, dst in specs:
                    pps4 = [psB.tile([P, IQW], F32, tag="S", name=f"pp{b}{name}{c}",
                                     bufs=6)
                            for c in range(IQ)]
                    for kk in range(D // P):
                        for c in range(IQ):
                            nc.tensor.matmul(
                                pps4[c][:M, :], w_t[:, kk, :M],
                                x_t[b][:, kk, c * IQW:(c + 1) * IQW],
                                start=(kk == 0), stop=(kk == D // P - 1))
                    for c in range(IQ):
                        sl = slice(c * IQW, (c + 1) * IQW)
                        if name == "q":
                            nc.scalar.mul(dst[:, sl], pps4[c][:, :], SCALE)
                        elif name == "m":
                            nc.scalar.activation(dst[:NH, sl], pps4[c][:NH, :],
                                                 mybir.ActivationFunctionType.Sigmoid)
                        else:
                            nc.scalar.copy(dst[:, sl], pps4[c][:, :])

                # RoPE on qT and kT (b0 now; b1 deferred)
                qt[b] = proj.tile([P, N], BF16, tag=f"qt{b}", name=f"qt{b}")
                kt[b] = proj.tile([P, N], BF16, tag=f"kt{b}", name=f"kt{b}")
                if late:
                    qt_raw1, kt_raw1 = qt_raw, kt_raw
                else:
                    for src, dst in ((qt_raw, qt[b]), (kt_raw, kt[b])):
                        rot_t = ptmp.tile([P, N], BF16, tag="rot")
                        for hh in range(NH):
                            lo = hh * DH
                            nc.vector.tensor_copy(rot_t[lo:lo + 32, :], src[lo + 32:lo + 64, :])
                            nc.vector.tensor_copy(rot_t[lo + 32:lo + 64, :], src[lo:lo + 32, :])
                        nc.vector.tensor_tensor(dst[:], src[:], cosT[:], mybir.AluOpType.mult)
                        nc.vector.tensor_tensor(rot_t[:], rot_t[:], sinT_rot[:], mybir.AluOpType.mult)
                        nc.vector.tensor_tensor(dst[:], dst[:], rot_t[:], mybir.AluOpType.add)

                # mix natural [128, NCH, NH] f32 via PE transposes; mixc = 1-mix
                mixn[b] = proj.tile([P, NCH, NH], F32, tag=f"mixn{b}", name=f"mixn{b}")
                mixc = proj.tile([P, NCH, NH], F32, tag=f"mixc{b}", name=f"mixc{b}")
                mixc_l[b] = mixc
                for t in range(NCH):
                    mps = psB.tile([P, DH], BF16, tag="T", bufs=2)
                    nc.tensor.matmul(mps[:, :NH], mixT[:NH, t * P:(t + 1) * P], ident_b[:NH, :NH],
                                     is_transpose=True, start=True, stop=True)
                    nc.scalar.copy(mixn[b][:, t, :], mps[:, :NH])
                nc.vector.tensor_scalar(mixc[:], mixn[b][:], -1.0, 1.0,
                                        mybir.AluOpType.mult, mybir.AluOpType.add)

                # v_aug: [j, ones(64) | v(64)]; the ones block replicates the
                # softmax denominator onto PSUM partitions 0:64 of oacc (the
                # custom reciprocal DVE op requires a base-0 input on HW).
                for hh in range(NH):
                    va = proj.tile([P, NCH, P], BF16, tag=f"va{b}{hh}", name=f"va{b}{hh}")
                    nc.gpsimd.memset(va[:, :, 0:DH], 1.0)
                    vr = vr_t[(b, hh)]
                    for t in range(NCH):
                        vps = psB.tile([P, DH], BF16, tag="T", bufs=2)
                        lo = hh * DH
                        nc.tensor.matmul(vps[:, :DH], vt[b][lo:lo + DH, t * P:(t + 1) * P],
                                         ident_b[lo:lo + DH, lo:lo + DH], is_transpose=True,
                                         start=True, stop=True)
                        if late:
                            # park the transposed v in SBUF; lerp runs later.
                            # On DVE: these copies are the last readers of the
                            # psB banks, and the ACT queue (q/k/v copies)
                            # drains ~10us later than the DVE here -- putting
                            # them on ACT delayed the attention-pool barrier
                            # and the first exp to ~104us (measured).
                            nc.vector.tensor_copy(vpsS[hh][:, t, :], vps[:, :DH])
                        else:
                            # va = v*(1-mix) + vr*mix
                            vrm = ptmp.tile([P, DH], BF16, tag="df")
                            nc.vector.tensor_scalar(vrm[:], vr[:, t, :],
                                                    mixn[b][:, t, hh:hh + 1],
                                                    None, mybir.AluOpType.mult)
                            nc.vector.scalar_tensor_tensor(va[:, t, DH:P], vps[:, :DH],
                                                           mixc[:, t, hh:hh + 1], vrm[:],
                                                           mybir.AluOpType.mult, mybir.AluOpType.add)
                    vaug[(b, hh)] = va
            pctx.close()

            # ---- b1's deferred rope + lerp, emitted as small thunks woven
            # into the b0-only first attention pass (fills the DVE while the
            # PE/ACT stream b0's scores+exp) ----
            def late_ops_gen():
                for src, dst in ((qt_raw1, qt[1]), (kt_raw1, kt[1])):
                    rot_t = blate.tile([P, N], BF16, tag="rot1", bufs=1,
                                       name=f"rot1_{src.name}")
                    for hh in range(NH):
                        lo = hh * DH
                        yield lambda r=rot_t, s=src, lo=lo: nc.vector.tensor_copy(
                            r[lo:lo + 32, :], s[lo + 32:lo + 64, :])
                        yield lambda r=rot_t, s=src, lo=lo: nc.vector.tensor_copy(
                            r[lo + 32:lo + 64, :], s[lo:lo + 32, :])
                    yield lambda d=dst, s=src: nc.vector.tensor_tensor(
                        d[:], s[:], cosT[:], mybir.AluOpType.mult)
                    yield lambda r=rot_t: nc.vector.tensor_tensor(
                        r[:], r[:], sinT_rot[:], mybir.AluOpType.mult)
                    yield lambda d=dst, r=rot_t: nc.vector.tensor_tensor(
                        d[:], d[:], r[:], mybir.AluOpType.add)
                for hh in range(NH):
                    for t in range(NCH):
                        def lerp(hh=hh, t=t):
                            vrm = blate.tile([P, DH], BF16, tag="vrm", bufs=2,
                                             name=f"vrm1_{hh}_{t}")
                            nc.vector.tensor_scalar(vrm[:], vr_t[(1, hh)][:, t, :],
                                                    mixn[1][:, t, hh:hh + 1],
                                                    None, mybir.AluOpType.mult)
                            nc.vector.scalar_tensor_tensor(
                                vaug[(1, hh)][:, t, DH:P], vpsS[hh][:, t, :],
                                mixc_l[1][:, t, hh:hh + 1], vrm[:],
                                mybir.AluOpType.mult, mybir.AluOpType.add)
                        yield lerp

            late_ops = deque(late_ops_gen())

            # ---- tail pools (allocated after x/transients are freed) ----
            # all 8 outTq tiles (2 per iq) stay live until the outproj tail
            otqp = ctx.enter_context(tc.tile_pool(name="otqp", bufs=8))
            finp = ctx.enter_context(tc.tile_pool(name="finp", bufs=3))
            zpool = ctx.enter_context(tc.tile_pool(name="zpool", bufs=4, side="right"))
            # attention PSUM pools live in their own stack so the outproj
            # tail can reclaim all 8 banks for a deeper ring
            actx = ExitStack()
            ps = actx.enter_context(tc.tile_pool(name="ps", bufs=2, space="PSUM"))
            oaccp = actx.enter_context(tc.tile_pool(name="oaccp", bufs=4, space="PSUM"))

            # ---- attention, streaming per i-chunk of 512 ----
            # Per (b, jt): two row-tiled CONCURRENT score MMs (h0 rows 0-63,
            # h1 rows 64-127) -> one [128,1024] PSUM pair -> one exp -> one
            # bias-mult -> two oacc MMs.  oacc rows 64:128 accumulate the
            # denominator via va's ones block.
            # oacc MMs trail the scores stream by DEPTH/2 (b,jt) groups so
            # their pT operand is long-ready when the PE reaches them; a
            # shallow trail head-of-line-blocks the next S-pair behind an
            # oacc waiting on exp+mult.  pT ring (6) bounds the trail.
            DEPTH = 8
            pending = deque()   # (oacc_tile, va, jt, pT, half)

            def flush_one():
                oa, va, jt_, pT_, half = pending.popleft()
                nc.tensor.matmul(oa[:], va[:, jt_, :], pT_[:, half * IQW:(half + 1) * IQW],
                                 start=(jt_ == 0), stop=(jt_ == JT - 1))

            def emit_norm_one(oa, otq_b, hh):
                # rz = 1/z from the replicated denominator rows (0:64); then
                # scale the v rows (64:128).
                rz = zpool.tile([DH, IQW], F32, tag="rz")
                nc.vector.reciprocal_approx_fast(rz[:], oa[0:DH, :])
                nc.vector.tensor_tensor(otq_b[hh * DH:(hh + 1) * DH, :], rz[:],
                                        oa[DH:P, :], mybir.AluOpType.mult)

            def emit_proj_unit(iq, b, it, outTq, pool, split):
                tg = iq * (IQW // P) + it
                pp = pool.tile([P, 2 * IQW], F32, tag="S", name=f"pp{iq}{b}{it}")
                for dfi in range(2):
                    nc.tensor.matmul(pp[:, dfi * IQW:(dfi + 1) * IQW],
                                     outTq[b][:, it * P:(it + 1) * P],
                                     wout_t[:, dfi * IQW:(dfi + 1) * IQW],
                                     start=True, stop=True)
                # fin copy (GPSIMD cannot read PSUM): in-stream units keep
                # the ACT queue clean (exp cadence) -> single DVE cast; tail
                # units split DVE/ACT halves to halve the tail latency.
                fin = finp.tile([P, D], BF16, tag="fin")
                if split:
                    nc.vector.tensor_copy(fin[:, 0:IQW], pp[:, 0:IQW])
                    nc.scalar.copy(fin[:, IQW:2 * IQW], pp[:, IQW:2 * IQW])
                else:
                    nc.vector.tensor_copy(fin[:], pp[:])
                nc.gpsimd.dma_start(out[b, tg], fin[:])

            norms_pending = deque()   # thunks; popped at jt 0/1 half-steps
            proj_units = deque()      # (iq, b, it, outTq)

            def emit_attn(iq, jt, b, seqbase, oacc_b, outTq_b):
                isl = slice(iq * IQW, (iq + 1) * IQW)
                bias_sb = bias_tiles[seqbase + jt]
                S = ps.tile([P, 2 * IQW], F32, tag="S", name=f"S{iq}_{jt}{b}")
                for hh in range(NH):
                    lo = hh * DH
                    nc.tensor.matmul(S[:, hh * IQW:(hh + 1) * IQW],
                                     kt[b][lo:lo + DH, jt * P:(jt + 1) * P],
                                     qt[b][lo:lo + DH, isl],
                                     start=True, stop=True)
                eS = esp.tile([P, 2 * IQW], BF16, tag="eS")
                nc.scalar.activation(eS[:], S[:], mybir.ActivationFunctionType.Exp)
                pT = ptp.tile([P, 2 * IQW], BF16, tag="pT")
                nc.vector.tensor_tensor(pT[:], bias_sb[:], eS[:], mybir.AluOpType.mult)
                for hh in range(NH):
                    pending.append((oacc_b[hh], vaug[(b, hh)], jt, pT, hh))
                while len(pending) > DEPTH:
                    flush_one()

            def push_norms(oacc_b, outTq_b):
                for hh in range(NH):
                    norms_pending.append(
                        lambda oa=oacc_b[hh], oq=outTq_b, hh=hh: emit_norm_one(oa, oq, hh))

            for iq in range(IQ):
                outTq = [otqp.tile([P, IQW], BF16, tag="otq", name=f"otq{iq}_{b}")
                         for b in range(B)]
                if iq == 0:
                    # pass1: b0-only attention over iq0, woven with b1's
                    # deferred rope/lerp DVE thunks
                    oacc_b0 = [oaccp.tile([P, IQW], F32, tag="oacc",
                                          name=f"oacc0_0{hh}") for hh in range(NH)]
                    for jt in range(JT):
                        issue_bias(jt + PF)
                        emit_attn(0, jt, 0, 0, oacc_b0, outTq[0])
                        for _ in range(3):
                            if late_ops:
                                late_ops.popleft()()
                    while pending:
                        flush_one()
                    while late_ops:
                        late_ops.popleft()()
                    push_norms(oacc_b0, outTq[0])
                    # pass2: b1 over iq0 (bias tiles re-streamed, seq 16..31)
                    oacc_b1 = [oaccp.tile([P, IQW], F32, tag="oacc",
                                          name=f"oacc0_1{hh}") for hh in range(NH)]
                    for jt in range(JT):
                        issue_bias(JT + jt + PF)
                        if jt in (1, 2) and norms_pending:
                            norms_pending.popleft()()
                        emit_attn(0, jt, 1, JT, oacc_b1, outTq[1])
                    while pending:
                        flush_one()
                    push_norms(oacc_b1, outTq[1])
                    oacc_pair = [oacc_b0, oacc_b1]
                elif iq < IQ - 1:
                    oacc_pair = [[oaccp.tile([P, IQW], F32, tag="oacc",
                                             name=f"oacc{iq}_{b}{hh}") for hh in range(NH)]
                                 for b in range(B)]
                    seqbase = JT + iq * JT
                    for jt in range(JT):
                        issue_bias(seqbase + jt + PF)
                        for b in range(B):
                            if jt in (1, 2) and norms_pending:
                                norms_pending.popleft()()
                            emit_attn(iq, jt, b, seqbase, oacc_pair[b], outTq[b])
                        # previous iq's out-projection rides the attention
                        # stream (one unit per jt) so its out-DMA overlaps
                        # compute instead of piling up in the tail
                        if jt >= 2 and proj_units and proj_units[0][0] < iq:
                            emit_proj_unit(*proj_units.popleft(), pool=ps, split=False)
                    while pending:
                        flush_one()
                    for b in range(B):
                        push_norms(oacc_pair[b], outTq[b])
                else:
                    # final iq split by batch: b0's norms + out-projection
                    # drain during b1's pass, shrinking the serial tail
                    oacc_b0 = [oaccp.tile([P, IQW], F32, tag="oacc",
                                          name=f"oacc{iq}_0{hh}") for hh in range(NH)]
                    seqbase = JT + iq * JT
                    for jt in range(JT):
                        issue_bias(seqbase + jt + PF)
                        if jt in (1, 2, 3, 4) and norms_pending:
                            norms_pending.popleft()()
                        emit_attn(iq, jt, 0, seqbase, oacc_b0, outTq[0])
                        if jt >= 4 and proj_units and proj_units[0][0] < iq:
                            emit_proj_unit(*proj_units.popleft(), pool=ps, split=False)
                    while pending:
                        flush_one()
                    push_norms(oacc_b0, outTq[0])
                    oacc_b1 = [oaccp.tile([P, IQW], F32, tag="oacc",
                                          name=f"oacc{iq}_1{hh}") for hh in range(NH)]
                    seqbase2 = JT + IQ * JT
                    for jt in range(JT):
                        issue_bias(seqbase2 + jt + PF)
                        if jt in (1, 2) and norms_pending:
                            norms_pending.popleft()()
                        emit_attn(iq, jt, 1, seqbase2, oacc_b1, outTq[1])
                        if jt == 2:
                            for it in range(IQW // P):
                                proj_units.append((iq, 0, it, outTq))
                        if jt >= 3 and proj_units:
                            emit_proj_unit(*proj_units.popleft(), pool=ps, split=False)
                    while pending:
                        flush_one()
                    push_norms(oacc_b1, outTq[1])
                    oacc_pair = [oacc_b0, oacc_b1]
                if iq < IQ - 1:
                    for b in range(B):
                        for it in range(IQW // P):
                            proj_units.append((iq, b, it, outTq))
                else:
                    for it in range(IQW // P):
                        proj_units.append((iq, 1, it, outTq))
            # all out-projections run as one pipelined tail: any unit placed
            # inside the attention stream steals an S-ring slot and stalls
            # the exp cadence ~1.2us (measured), 24x per run.  The attention
            # PSUM pools close first so the tail gets a 4-deep [128,1024]
            # ring (all 8 banks) -- with the 2-slot S-ring the tail ran
            # copy-serialized at ~2us/unit (65us, measured).
            while norms_pending:
                norms_pending.popleft()()
            actx.close()
            psT = ctx.enter_context(tc.tile_pool(name="psT", bufs=4, space="PSUM"))
            while proj_units:
                emit_proj_unit(*proj_units.popleft(), pool=psT, split=True)

    nc.compile()
    return nc


def make_in_maps(x, mask, rotary_emb, attn_bias, value_residual, Wq, Wkv, Wmix, Wout, bout):
    """Shard + lay out the full inputs for the 8 cores (bf16 staging)."""
    import ml_dtypes
    bf16 = ml_dtypes.bfloat16
    x = np.asarray(x); rotary_emb = np.asarray(rotary_emb)
    attn_bias = np.asarray(attn_bias); value_residual = np.asarray(value_residual)
    Wq = np.asarray(Wq); Wkv = np.asarray(Wkv); Wmix = np.asarray(Wmix)
    Wout = np.asarray(Wout); bout = np.asarray(bout)

    xt_pre = np.ascontiguousarray(
        x.transpose(0, 2, 1).reshape(B, D // P, P, N).transpose(0, 2, 1, 3)).astype(bf16)
    rott = np.ascontiguousarray(rotary_emb.T)

    def wslice(Wcols):  # [1024, 128 or NH] -> [128, 8, M]
        M = Wcols.shape[1]
        return np.ascontiguousarray(
            Wcols.reshape(D // P, P, M).transpose(1, 0, 2)).astype(bf16)

    in_maps = []
    for c in range(NC):
        h0 = NH * c
        hs = slice(h0, h0 + NH)
        # exp(bias) transposed to [h, j, i], then arranged so each (jt, iq)
        # tile is [128(j), h0-block(512) | h1-block(512)]
        biasT = np.exp(attn_bias[hs].transpose(0, 2, 1))  # [NH, j, i]
        biasPa = np.ascontiguousarray(
            biasT.reshape(NH, JT, P, IQ, IQW).transpose(1, 3, 2, 0, 4)
            .reshape(JT, IQ, P, NH * IQW)).astype(bf16)
        vrp = np.ascontiguousarray(
            value_residual[:, hs].reshape(B, NH, NCH, P, DH).transpose(0, 1, 3, 2, 4)).astype(bf16)
        in_maps.append({
            "xt": xt_pre,
            "wq": wslice(Wq[:, h0 * DH:(h0 + NH) * DH]),
            "wk": wslice(Wkv[:, h0 * DH:(h0 + NH) * DH]),
            "wv": wslice(Wkv[:, H * DH + h0 * DH: H * DH + (h0 + NH) * DH]),
            "wmix": wslice(Wmix[:, hs]),
            "wout": np.ascontiguousarray(Wout[h0 * DH:(h0 + NH) * DH, :]).astype(bf16),
            "rott": rott,
            "biasP": biasPa,
            "vrp": vrp,
        })
    return in_maps


def unshard(results, bout):
    full = np.zeros((B, NCH, P, D), np.float32)
    for r in results:
        full += r["out"].astype(np.float32)
    return full.reshape(B, N, D) + np.asarray(bout, np.float32)


_NC_CACHE = None


def kernel(**inputs):
    global _NC_CACHE
    from concourse.bass_utils import run_bass_kernel_spmd
    if _NC_CACHE is None:
        _NC_CACHE = build_nc()
    in_maps = make_in_maps(**inputs)
    res = run_bass_kernel_spmd(_NC_CACHE, in_maps, core_ids=list(range(NC)))
    return unshard(res.results, inputs["bout"])
